# revision 7
# baseline (speedup 1.0000x reference)
"""Trainium2 Bass kernel for nn_MultiHeadAttention_68152541053005.

Multi-head attention (B=2, N=2048, D=1024, H=16, d=64) with RoPE,
per-head RMSNorm on q/k, per-dim scale on q, causal softmax.

Sharding: 8 cores = 2 batch groups x 4 head-groups (4 heads/core).
Each core computes QKV projection for its 4 heads on its batch,
attention, and a partial output projection; the host sums the 4
partial outputs per batch (equivalent to the all-reduce after the
output projection).

Per-core kernel (S.T orientation flash-style attention):
  - x.T tiles via DMA-transpose (bf16) or PE transpose (f32 variants)
  - QKV projection: psum[tok,768] accumulated over 8 D-chunks
  - fused postproc: RMSNorm stats from pre-RoPE q/k (rotation preserves
    the per-pair norm), RoPE via host-folded tables (per-dim scales
    folded in), rsqrt via ACT ln/exp (stays in the exp table set)
  - PE transpose q/k -> qT/kT [head*64, tok]
  - attention per (head, q-block of 512): S.T[k128, q512] = kT.T @ qT,
    exp on ACT (no max subtraction; scores are O(1) by construction),
    causal masking post-exp on diagonal blocks, ctx.T accumulation with
    a ones-augmented v (denominator rides along as 64 extra psum rows)
  - normalize ctx.T by DVE reciprocal, output projection -> out.T
"""

import os
import sys

if "/opt/trn_rl_repo" not in sys.path:
    sys.path.insert(0, "/opt/trn_rl_repo")

import numpy as np
from contextlib import ExitStack

import concourse.bacc as bacc
import concourse.bass as bass
import concourse.mybir as mybir
import concourse.tile as tile

AP = bass.AP
F32 = mybir.dt.float32
F32R = mybir.dt.float32r
BF16 = mybir.dt.bfloat16
AFT = mybir.ActivationFunctionType

B, N, D, H, HD = 2, 2048, 1024, 16, 64
NH = 4            # heads per core
HALF = HD // 2    # 32
TC = N // 128     # 16 token chunks
DC = D // 128     # 8 D chunks
QB = N // 512     # 4 q blocks
LOG2_E = 1.442695041
RMS_EPS = 1e-6
MAX_TIMESCALE = 10000.0

VARIANT = os.environ.get("MHA_VARIANT", "f32r")  # bf16 | f32r | f32


def _dt_np(dt):
    import ml_dtypes
    return np.dtype(ml_dtypes.bfloat16) if dt == BF16 else np.float32


class Cfg:
    def __init__(self, variant):
        self.variant = variant
        if variant == "bf16":
            self.dt_x = BF16    # x in DRAM
            self.dt_w = BF16    # weights
            self.dt_mm = BF16   # staged matmul operands (qT/kT/v/PT/ctxT)
            self.mm_cast = None  # matmul-time bitcast
        elif variant == "f32r":
            self.dt_x = F32R
            self.dt_w = F32R
            self.dt_mm = F32R
            self.mm_cast = None
        else:
            self.dt_x = F32
            self.dt_w = F32
            self.dt_mm = F32
            self.mm_cast = None


def _mm(ap, cfg):
    """View an operand AP in the matmul dtype (f32r bitcast when set)."""
    if cfg.mm_cast is not None:
        return ap.bitcast(cfg.mm_cast)
    return ap


def build_nc(cfg):
    nc = bacc.Bacc("TRN2", target_bir_lowering=False, debug=False)

    x_d = nc.dram_tensor("x", [N, D], cfg.dt_x, kind="ExternalInput")
    wqkv_d = nc.dram_tensor("wqkv", [D, 3 * NH * HD], cfg.dt_w, kind="ExternalInput")
    wo_d = nc.dram_tensor("wo", [NH * HD, D], cfg.dt_w, kind="ExternalInput")
    qtab_d = nc.dram_tensor("qtab", [N, 4 * HALF], F32, kind="ExternalInput")
    ktab_d = nc.dram_tensor("ktab", [N, 4 * HALF], F32, kind="ExternalInput")
    masks_d = nc.dram_tensor("masks", [128, 4 * 512], cfg.dt_mm, kind="ExternalInput")
    ident_d = nc.dram_tensor("ident", [128, 128], cfg.dt_mm, kind="ExternalInput")
    ones_d = nc.dram_tensor("ones", [128, NH * HD], cfg.dt_mm, kind="ExternalInput")
    outT_d = nc.dram_tensor("outT", [D, N], F32, kind="ExternalOutput")

    with tile.TileContext(nc) as tc, ExitStack() as ctx:
        build_tile_kernel(ctx, tc, cfg,
                          x_d.ap(), wqkv_d.ap(), wo_d.ap(), qtab_d.ap(),
                          ktab_d.ap(), masks_d.ap(), ident_d.ap(), ones_d.ap(),
                          outT_d.ap())
    nc.compile()
    return nc


def build_tile_kernel(ctx, tc, cfg, x, wqkv, wo, qtab, ktab, masks,
                      identD, onesD, outT):
    nc = tc.nc
    dt_mm = cfg.dt_mm

    res = ctx.enter_context(tc.tile_pool(name="res", bufs=1))
    stream = ctx.enter_context(tc.tile_pool(name="stream", bufs=3))
    scratch = ctx.enter_context(tc.tile_pool(name="scratch", bufs=3))

    # ---- resident constants -------------------------------------------------
    wqkv_sb = res.tile([128, DC * 768], cfg.dt_w, tag="wqkv")
    for c in range(DC):
        nc.sync.dma_start(wqkv_sb[:, 768 * c:768 * (c + 1)],
                          wqkv[128 * c:128 * (c + 1), :])
    wo_sb = res.tile([128, 2 * D], cfg.dt_w, tag="wo")
    for r in range(2):
        nc.sync.dma_start(wo_sb[:, D * r:D * (r + 1)],
                          wo[128 * r:128 * (r + 1), :])
    masks_sb = res.tile([128, 4 * 512], dt_mm, tag="masks")
    nc.sync.dma_start(masks_sb[:], masks[:])
    eps_sb = res.tile([128, 1], F32, tag="eps")
    nc.vector.memset(eps_sb[:], RMS_EPS)
    ident = res.tile([128, 128], dt_mm, tag="ident")
    nc.sync.dma_start(ident[:], identD[:])

    # resident activations
    qT = [res.tile([128, N], dt_mm, tag=f"qT{i}", name=f"qT{i}") for i in range(2)]  # heads 01 / 23
    kT = [res.tile([128, N], dt_mm, tag=f"kT{i}", name=f"kT{i}") for i in range(2)]
    ctxT = [res.tile([128, N], dt_mm, tag=f"ctxT{i}", name=f"ctxT{i}") for i in range(2)]
    v_stage = [res.tile([128, 512], dt_mm, tag=f"vst{t}", name=f"vst{t}") for t in range(TC)]
    for t in range(TC):
        va = v_stage[t][:]
        ones_dst = AP(va.tensor, va.offset + HD, [va.ap[0], [128, NH], [1, HD]])
        nc.sync.dma_start(ones_dst, onesD[:])

    # x.T: bf16 -> resident via DMA transpose; f32 -> streamed PE transpose
    if cfg.variant == "bf16":
        xT_big = [res.tile([128, N], cfg.dt_x, tag=f"xT{c}", name=f"xTbig{c}") for c in range(DC)]
        for c in range(DC):
            nc.sync.dma_start(xT_big[c][:], x[:, 128 * c:128 * (c + 1)],
                              transpose=True)

    # ---- phase A: QKV projection + postproc + q/k transposes ---------------
    with tc.tile_pool(name="psA", bufs=2, space="PSUM") as psA, \
         tc.tile_pool(name="psT", bufs=2, space="PSUM") as psT:
        for t in range(TC):
            trow = slice(128 * t, 128 * (t + 1))
            if cfg.variant == "bf16":
                def xT_ap(c, _t=t):
                    return xT_big[c][:, 128 * _t:128 * (_t + 1)]
            else:
                x_t = stream.tile([128, D], cfg.dt_x, tag="x_in")
                nc.sync.dma_start(x_t[:], x[trow, :])
                xT_t = stream.tile([128, D], cfg.dt_x, tag="xT")
                for g in range(2):
                    pT = psT.tile([128, 512], cfg.dt_x, tag="xTp")
                    for c4 in range(4):
                        c = 4 * g + c4
                        nc.tensor.transpose(pT[:, 128 * c4:128 * (c4 + 1)],
                                            x_t[:, 128 * c:128 * (c + 1)],
                                            ident[:])
                    nc.scalar.copy(xT_t[:, 512 * g:512 * (g + 1)], pT[:])

                def xT_ap(c, _x=xT_t):
                    return _x[:, 128 * c:128 * (c + 1)]

            # qkv psum: [tok 128, 768] = q(256) | k(256) | v(256)
            pqkv = psA.tile([128, 768], F32, tag="pqkv")
            for c in range(DC):
                lhsT = _mm(xT_ap(c), cfg)
                nc.tensor.matmul(pqkv[:, 0:512],
                                 lhsT, _mm(wqkv_sb[:, 768 * c:768 * c + 512], cfg),
                                 start=(c == 0), stop=(c == DC - 1))
                nc.tensor.matmul(pqkv[:, 512:768],
                                 lhsT, _mm(wqkv_sb[:, 768 * c + 512:768 * (c + 1)], cfg),
                                 start=(c == 0), stop=(c == DC - 1))

            # --- postproc ---
            # single psum->sbuf evict (HW: max one PSUM input per DVE op)
            qk_sb = scratch.tile([128, 512], F32, tag="qk_sb")
            nc.vector.tensor_copy(qk_sb[:], pqkv[:, 0:512])
            # sumsq per head: q heads 0..3 then k heads 0..3 -> rs [128, 8]
            sq = scratch.tile([128, 512], F32, tag="sq")
            nc.vector.tensor_mul(sq[:], qk_sb[:], qk_sb[:])
            ssq = scratch.tile([128, 8], F32, tag="ssq")
            nc.vector.reduce_sum(ssq[:],
                                 sq[:].rearrange("p (h d) -> p h d", d=HD),
                                 axis=mybir.AxisListType.X)
            lnv = scratch.tile([128, 8], F32, tag="lnv")
            nc.scalar.activation(lnv[:], ssq[:], AFT.Ln, bias=eps_sb[:],
                                 scale=1.0 / HD)
            rs = scratch.tile([128, 8], F32, tag="rs")
            nc.scalar.activation(rs[:], lnv[:], AFT.Exp, scale=-0.5)

            qtab_t = stream.tile([128, 4 * HALF], F32, tag="qtab")
            nc.sync.dma_start(qtab_t[:], qtab[trow, :])
            ktab_t = stream.tile([128, 4 * HALF], F32, tag="ktab")
            nc.sync.dma_start(ktab_t[:], ktab[trow, :])

            def rope(base, tab_t, rs_off, out_tile):
                # halves of each head: AP [128, (64,NH), 32] at offset
                def halfap(off):
                    a = qk_sb[:]
                    return AP(a.tensor, a.offset + base + off,
                              [a.ap[0], [HD, NH], [1, HALF]])

                def tabap(k):
                    a = tab_t[:]
                    return AP(a.tensor, a.offset + HALF * k,
                              [a.ap[0], [0, NH], [1, HALF]])

                tmp = [scratch.tile([128, NH * HALF], F32, tag=f"rp{i}",
                                     name=f"rp{i}") for i in range(4)]

                def tview(i):
                    a = tmp[i][:]
                    return AP(a.tensor, a.offset, [a.ap[0], [HALF, NH], [1, HALF]])

                q1, q2 = halfap(0), halfap(HALF)
                nc.vector.tensor_mul(tview(0), q1, tabap(0))       # q1*cos_a
                nc.vector.tensor_mul(tview(1), q2, tabap(1))       # q2*sin_a
                nc.vector.tensor_mul(tview(2), q2, tabap(2))       # q2*cos_b
                nc.vector.tensor_mul(tview(3), q1, tabap(3))       # q1*sin_b
                roped = scratch.tile([128, NH * HD], F32, tag="roped")

                def rview(off):
                    a = roped[:]
                    return AP(a.tensor, a.offset + off,
                              [a.ap[0], [HD, NH], [1, HALF]])

                nc.vector.tensor_sub(rview(0), tview(0), tview(1))
                nc.vector.tensor_add(rview(HALF), tview(2), tview(3))
                # scale by rs (broadcast along d)
                a = rs[:]
                rs_b = AP(a.tensor, a.offset + rs_off, [a.ap[0], [1, NH], [0, HD]])
                rfull = roped[:].rearrange("p (h d) -> p h d", d=HD)
                ofull = out_tile[:].rearrange("p (h d) -> p h d", d=HD)
                nc.vector.tensor_mul(ofull, rfull, rs_b)

            q_stage = scratch.tile([128, NH * HD], dt_mm, tag="qstage")
            k_stage = scratch.tile([128, NH * HD], dt_mm, tag="kstage")
            rope(0, qtab_t, 0, q_stage)
            rope(256, ktab_t, 4, k_stage)

            # v -> v_stage [v_h0|ones|v_h1|ones|...]
            vs = v_stage[t]
            va = vs[:]
            v_dst = AP(va.tensor, va.offset, [va.ap[0], [128, NH], [1, HD]])
            pa = pqkv[:]
            v_src = AP(pa.tensor, pa.offset + 512, [pa.ap[0], [HD, NH], [1, HD]])
            nc.vector.tensor_copy(v_dst, v_src)

            # transposes q/k [128,256] -> qT/kT at token columns
            for src, dsts in ((q_stage, qT), (k_stage, kT)):
                for i in range(2):
                    pt = psT.tile([128, 128], dt_mm, tag="qkT")
                    nc.tensor.transpose(pt[:], src[:, 128 * i:128 * (i + 1)],
                                        ident[:])
                    nc.vector.tensor_copy(dsts[i][:, trow], pt[:])

    # ---- phase B/C: attention + output projection --------------------------
    with tc.tile_pool(name="psS", bufs=3, space="PSUM") as psS, \
         tc.tile_pool(name="psC", bufs=2, space="PSUM") as psC, \
         tc.tile_pool(name="psO", bufs=2, space="PSUM") as psO, \
         tc.tile_pool(name="ptp", bufs=4) as ptp:
        for Q in range(QB):
            qcol = slice(512 * Q, 512 * (Q + 1))
            for h in range(NH):
                g, off = divmod(h, 2)
                row = slice(64 * off, 64 * off + 64)
                nkb = 4 * Q + 4
                pctx = psC.tile([128, 512], F32, tag="ctx")
                for j in range(nkb):
                    pst = psS.tile([128, 512], F32, tag="st")
                    nc.tensor.matmul(
                        pst[:],
                        _mm(kT[g][row, 128 * j:128 * (j + 1)], cfg),
                        _mm(qT[g][row, qcol], cfg),
                        start=True, stop=True)
                    pt = ptp.tile([128, 512], dt_mm, tag="pt")
                    nc.scalar.activation(pt[:], pst[:], AFT.Exp)
                    o = j - 4 * Q
                    if o >= 0:
                        nc.vector.tensor_mul(
                            pt[:], pt[:], masks_sb[:, 512 * o:512 * (o + 1)])
                    nc.tensor.matmul(
                        pctx[:],
                        _mm(v_stage[j][:, 128 * h:128 * (h + 1)], cfg),
                        _mm(pt[:], cfg),
                        start=(j == 0), stop=(j == nkb - 1))
                recip = scratch.tile([64, 512], F32, tag="recip")
                nc.vector.reciprocal(recip[:], pctx[64:128, :])
                nc.vector.tensor_mul(ctxT[g][row, qcol], pctx[0:64, :], recip[:])

            # output projection for this q block
            for m in range(DC):
                po = psO.tile([128, 512], F32, tag="po")
                for r in range(2):
                    nc.tensor.matmul(
                        po[:],
                        _mm(wo_sb[:, D * r + 128 * m:D * r + 128 * (m + 1)], cfg),
                        _mm(ctxT[r][:, qcol], cfg),
                        start=(r == 0), stop=(r == 1))
                ob = scratch.tile([128, 512], F32, tag="ob")
                nc.scalar.copy(ob[:], po[:])
                nc.sync.dma_start(outT[128 * m:128 * (m + 1), qcol], ob[:])


# ---------------------------------------------------------------------------
# host side
# ---------------------------------------------------------------------------

_CACHE = {}


def _get_nc(cfg):
    key = cfg.variant
    if key not in _CACHE:
        _CACHE[key] = build_nc(cfg)
    return _CACHE[key]


def _host_tables(q_ln_scale, k_ln_scale, per_dim_scale):
    frac = 2.0 * np.arange(HALF, dtype=np.float32) / HD
    ts = (MAX_TIMESCALE ** frac).astype(np.float32)
    pos = np.arange(N, dtype=np.float32)
    sinu = pos[:, None] / ts[None, :]
    SIN = np.sin(sinu).astype(np.float32)
    COS = np.cos(sinu).astype(np.float32)
    qs = (LOG2_E / np.sqrt(np.float32(HD))
          * np.logaddexp(0.0, per_dim_scale.astype(np.float64))).astype(np.float32)
    qscale = (q_ln_scale * qs).astype(np.float32)
    kscale = k_ln_scale.astype(np.float32)

    def pack(scale):
        return np.concatenate(
            [COS * scale[None, :HALF], SIN * scale[None, :HALF],
             COS * scale[None, HALF:], SIN * scale[None, HALF:]],
            axis=1).astype(np.float32)

    return pack(qscale), pack(kscale)


def _host_masks(dt_np_):
    # mask_o[r, c] = 1 if c >= r + 128*o  (S.T block: rows k, cols q)
    r = np.arange(128)[:, None]
    c = np.arange(512)[None, :]
    ms = [(c >= r + 128 * o).astype(np.float32) for o in range(4)]
    return np.concatenate(ms, axis=1).astype(dt_np_)


def kernel(**inputs):
    from concourse.bass_utils import run_bass_kernel_spmd

    cfg = Cfg(VARIANT)
    nc = _get_nc(cfg)

    x = np.asarray(inputs["inputs_q"], dtype=np.float32)
    wq = np.asarray(inputs["wq"], dtype=np.float32)
    wk = np.asarray(inputs["wk"], dtype=np.float32)
    wv = np.asarray(inputs["wv"], dtype=np.float32)
    wo = np.asarray(inputs["wo"], dtype=np.float32)

    qtab, ktab = _host_tables(np.asarray(inputs["q_ln_scale"], np.float32),
                              np.asarray(inputs["k_ln_scale"], np.float32),
                              np.asarray(inputs["per_dim_scale"], np.float32))
    dtw_np = _dt_np(cfg.dt_w)
    dtx_np = _dt_np(cfg.dt_x)
    masks = _host_masks(_dt_np(cfg.dt_mm))

    in_maps = []
    for c in range(8):
        b, g = divmod(c, 4)
        hs = slice(NH * g, NH * (g + 1))
        wqkv_c = np.concatenate(
            [wq[:, hs, :].reshape(D, NH * HD),
             wk[:, hs, :].reshape(D, NH * HD),
             wv[:, hs, :].reshape(D, NH * HD)], axis=1)
        in_maps.append({
            "x": np.ascontiguousarray(x[b]).astype(dtx_np),
            "wqkv": np.ascontiguousarray(wqkv_c).astype(dtw_np),
            "wo": np.ascontiguousarray(wo[hs].reshape(NH * HD, D)).astype(dtw_np),
            "qtab": qtab, "ktab": ktab, "masks": masks,
            "ident": np.eye(128, dtype=_dt_np(cfg.dt_mm)),
            "ones": np.ones((128, NH * HD), dtype=_dt_np(cfg.dt_mm)),
        })

    trace = os.environ.get("MHA_TRACE", "0") == "1"
    res = run_bass_kernel_spmd(nc, in_maps, list(range(8)), trace=trace)
    if trace:
        kernel.last_exec_time_ns = res.exec_time_ns
        kernel.last_results = res

    out = np.zeros((B, N, D), dtype=np.float32)
    for c in range(8):
        out[c // 4] += res.results[c]["outT"].T
    return out


# revision 18
# speedup vs baseline: 1.4984x; 1.4984x over previous
"""Trainium2 Bass kernel for nn_MultiHeadAttention_68152541053005.

Multi-head attention (B=2, N=2048, D=1024, H=16, d=64) with RoPE,
per-head RMSNorm on q/k, per-dim scale on q, causal softmax.

Sharding: 8 cores = 2 batch groups x 4 head-groups (4 heads/core).
Each core computes QKV projection for its 4 heads on its batch,
attention, and a partial output projection; the host sums the 4
partial outputs per batch (equivalent to the all-reduce after the
output projection).

Per-core kernel (S.T orientation flash-style attention):
  - x.T tiles via DMA-transpose (bf16) or PE transpose (f32 variants)
  - QKV projection: psum[tok,768] accumulated over 8 D-chunks
  - fused postproc: RMSNorm stats from pre-RoPE q/k (rotation preserves
    the per-pair norm), RoPE via host-folded tables (per-dim scales
    folded in), rsqrt via ACT ln/exp (stays in the exp table set)
  - PE transpose q/k -> qT/kT [head*64, tok]
  - attention per (head, q-block of 512): S.T[k128, q512] = kT.T @ qT,
    exp on ACT (no max subtraction; scores are O(1) by construction),
    causal masking post-exp on diagonal blocks, ctx.T accumulation with
    a ones-augmented v (denominator rides along as 64 extra psum rows)
  - normalize ctx.T by DVE reciprocal, output projection -> out.T
"""

import os
import sys

if "/opt/trn_rl_repo" not in sys.path:
    sys.path.insert(0, "/opt/trn_rl_repo")

import numpy as np
from contextlib import ExitStack

import concourse.bacc as bacc
import concourse.bass as bass
import concourse.mybir as mybir
import concourse.tile as tile

AP = bass.AP
F32 = mybir.dt.float32
F32R = mybir.dt.float32r
BF16 = mybir.dt.bfloat16
AFT = mybir.ActivationFunctionType

B, N, D, H, HD = 2, 2048, 1024, 16, 64
NH = 4            # heads per core
HALF = HD // 2    # 32
TC = N // 128     # 16 token chunks
DC = D // 128     # 8 D chunks
QB = N // 512     # 4 q blocks
LOG2_E = 1.442695041
RMS_EPS = 1e-6
MAX_TIMESCALE = 10000.0

VARIANT = os.environ.get("MHA_VARIANT", "mix")  # bf16 | f32r | f32 | mix


def _dt_np(dt):
    import ml_dtypes
    return np.dtype(ml_dtypes.bfloat16) if dt == BF16 else np.float32


class Cfg:
    def __init__(self, variant):
        self.variant = variant
        if variant == "bf16":
            self.dt_x = BF16    # x in DRAM
            self.dt_w = BF16    # qkv weights
            self.dt_mm = BF16   # q/k path (qT/kT, stages)
            self.dt_v = BF16    # value path (PT/vt/ctxT/masks)
            self.dt_wo = BF16   # output projection weights
        elif variant == "f32r":
            self.dt_x = F32R
            self.dt_w = F32R
            self.dt_mm = F32R
            self.dt_v = F32R
            self.dt_wo = F32R
        elif variant == "mix":
            # f32r where precision matters (projection, q/k, scores),
            # bf16 on the post-softmax value path (probs are in [0,1])
            self.dt_x = F32R
            self.dt_w = F32R
            self.dt_mm = F32R
            self.dt_v = BF16
            self.dt_wo = BF16
        else:
            self.dt_x = F32
            self.dt_w = F32
            self.dt_mm = F32
            self.dt_v = F32
            self.dt_wo = F32
        self.mm_cast = None


def _mm(ap, cfg):
    """View an operand AP in the matmul dtype (f32r bitcast when set)."""
    if cfg.mm_cast is not None:
        return ap.bitcast(cfg.mm_cast)
    return ap


def build_nc(cfg):
    nc = bacc.Bacc("TRN2", target_bir_lowering=False, debug=False)

    x_d = nc.dram_tensor("x", [N, D], cfg.dt_x, kind="ExternalInput")
    wqkv_d = nc.dram_tensor("wqkv", [D, 3 * NH * HD], cfg.dt_w, kind="ExternalInput")
    wo_d = nc.dram_tensor("wo", [NH * HD, D], cfg.dt_wo, kind="ExternalInput")
    ctab_d = nc.dram_tensor("ctab", [N, 8 * HALF], F32, kind="ExternalInput")
    masks_d = nc.dram_tensor("masks", [128, 4 * 512], cfg.dt_v, kind="ExternalInput")
    ident_d = nc.dram_tensor("ident", [128, 128], cfg.dt_mm, kind="ExternalInput")
    ones_d = nc.dram_tensor("ones", [128, NH * HD], cfg.dt_v, kind="ExternalInput")
    outT_d = nc.dram_tensor("outT", [D, N], F32, kind="ExternalOutput")

    with tile.TileContext(nc) as tc, ExitStack() as ctx:
        build_tile_kernel(ctx, tc, cfg,
                          x_d.ap(), wqkv_d.ap(), wo_d.ap(), ctab_d.ap(),
                          masks_d.ap(), ident_d.ap(), ones_d.ap(),
                          outT_d.ap())
    nc.compile()
    return nc


def build_tile_kernel(ctx, tc, cfg, x, wqkv, wo, ctab, masks,
                      identD, onesD, outT):
    nc = tc.nc
    dt_mm = cfg.dt_mm

    res = ctx.enter_context(tc.tile_pool(name="res", bufs=1))
    stream = ctx.enter_context(tc.tile_pool(name="stream", bufs=3))
    scratch = ctx.enter_context(tc.tile_pool(name="scratch", bufs=2))
    qkpool = ctx.enter_context(tc.tile_pool(name="qkpool", bufs=9))
    ptp = ctx.enter_context(tc.tile_pool(name="ptp", bufs=3))

    # ---- resident constants ----
    wqkv_sb = res.tile([128, DC * 768], cfg.dt_w, tag="wqkv")
    for c in range(DC):
        nc.sync.dma_start(wqkv_sb[:, 768 * c:768 * (c + 1)],
                          wqkv[128 * c:128 * (c + 1), :])
    wo_sb = res.tile([128, 2 * D], cfg.dt_wo, tag="wo")
    for r in range(2):
        nc.sync.dma_start(wo_sb[:, D * r:D * (r + 1)],
                          wo[128 * r:128 * (r + 1), :])
    masks_sb = res.tile([128, 4 * 512], cfg.dt_v, tag="masks")
    nc.sync.dma_start(masks_sb[:], masks[:])
    ident = res.tile([128, 128], dt_mm, tag="ident")
    nc.sync.dma_start(ident[:], identD[:])

    qkT_all = res.tile([128, 4 * N], dt_mm, tag="qkT_all")
    qT = [qkT_all[:, i * N:(i + 1) * N] for i in range(2)]
    kT = [qkT_all[:, (2 + i) * N:(3 + i) * N] for i in range(2)]
    ctxT = [res.tile([128, N], cfg.dt_v, tag=f"ctxT{i}", name=f"ctxT{i}")
            for i in range(2)]
    vt = res.tile([128, TC * NH * 65], cfg.dt_v, tag="vt")
    va = vt[:]
    ones_dst = AP(va.tensor, va.offset + HD,
                  [va.ap[0], [NH * 65, TC], [65, NH], [1, 1]])
    nc.sync.dma_start(ones_dst, onesD[:, 0:TC * NH].rearrange(
        "p (t h) -> p t h", h=NH))

    if cfg.variant == "bf16":
        xT_big = [res.tile([128, N], cfg.dt_x, tag=f"xT{c}", name=f"xTbig{c}")
                  for c in range(DC)]
        for c in range(DC):
            nc.sync.dma_start(xT_big[c][:], x[:, 128 * c:128 * (c + 1)],
                              transpose=True)

    GROUPS = [range(0, TC // 2), range(TC // 2, TC)]
    xT_tiles = {}
    qk_sbs = {}

    def emit_xload(t, psT):
        if cfg.variant == "bf16":
            return None
        trow = slice(128 * t, 128 * (t + 1))
        x_t = stream.tile([128, D], cfg.dt_x, tag="x_in", name=f"x_in{t}")
        nc.sync.dma_start(x_t[:], x[trow, :])
        xT_t = stream.tile([128, D], cfg.dt_x, tag="xT", name=f"xT{t}")
        for g in range(2):
            pT = psT.tile([128, 512], cfg.dt_x, tag="xTp", name=f"xTp{t}_{g}")
            for c4 in range(4):
                c = 4 * g + c4
                nc.tensor.transpose(pT[:, 128 * c4:128 * (c4 + 1)],
                                    x_t[:, 128 * c:128 * (c + 1)], ident[:])
            nc.scalar.copy(xT_t[:, 512 * g:512 * (g + 1)], pT[:])
        xT_tiles[t] = xT_t

    def xT_ap(t, c):
        if cfg.variant == "bf16":
            return xT_big[c][:, 128 * t:128 * (t + 1)]
        return xT_tiles[t][:, 128 * c:128 * (c + 1)]

    def emit_groupA(gi, psA, psT):
        group = GROUPS[gi]
        G = len(group)
        if gi == 0 and cfg.variant != "bf16":
            emit_xload(0, psT)
        ssq_g = scratch.tile([128, 8 * G], F32, tag="ssq_g", name=f"ssq{gi}")
        for dt_i, t in enumerate(group):
            if t + 1 < TC and cfg.variant != "bf16":
                emit_xload(t + 1, psT)
            pqkv = psA.tile([128, 768], F32, tag="pqkv", name=f"pqkv{t}")
            for c in range(DC):
                lhsT = xT_ap(t, c)
                nc.tensor.matmul(pqkv[:, 0:512],
                                 lhsT, wqkv_sb[:, 768 * c:768 * c + 512],
                                 start=(c == 0), stop=(c == DC - 1))
                nc.tensor.matmul(pqkv[:, 512:768],
                                 lhsT, wqkv_sb[:, 768 * c + 512:768 * (c + 1)],
                                 start=(c == 0), stop=(c == DC - 1))
            qk_sb = qkpool.tile([128, 512], F32, tag="qk_sb", name=f"qk_sb{t}")
            nc.scalar.copy(qk_sb[:], pqkv[:, 0:512])
            qk_sbs[t] = qk_sb
            va2 = vt[:]
            v_dst = AP(va2.tensor, va2.offset + NH * 65 * t,
                       [va2.ap[0], [65, NH], [1, HD]])
            pa = pqkv[:]
            v_src = AP(pa.tensor, pa.offset + 512,
                       [pa.ap[0], [HD, NH], [1, HD]])
            nc.scalar.copy(v_dst, v_src)
            sq = scratch.tile([128, 512], F32, tag="sq")
            nc.vector.tensor_mul(sq[:], qk_sb[:], qk_sb[:])
            nc.vector.reduce_sum(ssq_g[:, 8 * dt_i:8 * (dt_i + 1)],
                                 sq[:].rearrange("p (h d) -> p h d", d=HD),
                                 axis=mybir.AxisListType.X)
        lnv_g = scratch.tile([128, 8 * G], F32, tag="lnv_g", name=f"lnv{gi}")
        eps_sb = scratch.tile([128, 1], F32, tag="eps")
        nc.vector.memset(eps_sb[:], RMS_EPS)
        nc.scalar.activation(lnv_g[:], ssq_g[:], AFT.Ln, bias=eps_sb[:],
                             scale=1.0 / HD)
        rs_g = scratch.tile([128, 8 * G], F32, tag="rs_g", name=f"rs{gi}")
        nc.scalar.activation(rs_g[:], lnv_g[:], AFT.Exp, scale=-0.5)

        for dt_i, t in enumerate(group):
            trow = slice(128 * t, 128 * (t + 1))
            qk_sb = qk_sbs.pop(t)
            ctab_t = stream.tile([128, 256], F32, tag="ctab")
            nc.sync.dma_start(ctab_t[:], ctab[trow, :])

            def dat(off, tile=qk_sb):
                a = tile[:]
                return AP(a.tensor, a.offset + off,
                          [a.ap[0], [256, 2], [HD, NH], [1, HALF]])

            def tab(f):
                a = ctab_t[:]
                return AP(a.tensor, a.offset + 64 * f,
                          [a.ap[0], [HALF, 2], [0, NH], [1, HALF]])

            tmp = [scratch.tile([128, 256], F32, tag=f"rp{i}",
                                name=f"rp{i}") for i in range(4)]
            roped = scratch.tile([128, 512], F32, tag="roped")
            nc.vector.tensor_mul(tmp[0][:], dat(0), tab(0))
            nc.vector.tensor_mul(tmp[1][:], dat(HALF), tab(1))
            nc.vector.tensor_sub(dat(0, roped), tmp[0][:], tmp[1][:])
            nc.vector.tensor_mul(tmp[2][:], dat(HALF), tab(2))
            nc.vector.tensor_mul(tmp[3][:], dat(0), tab(3))
            nc.vector.tensor_add(dat(HALF, roped), tmp[2][:], tmp[3][:])

            qk_stage = scratch.tile([128, 512], dt_mm, tag="qk_stage")
            ra = rs_g[:]
            rs_b = AP(ra.tensor, ra.offset + 8 * dt_i,
                      [ra.ap[0], [1, 8], [0, HD]])
            nc.vector.tensor_mul(
                qk_stage[:].rearrange("p (h d) -> p h d", d=HD),
                roped[:].rearrange("p (h d) -> p h d", d=HD), rs_b)

            ptq = psT.tile([128, 512], dt_mm, tag="qkT")
            for i in range(4):
                nc.tensor.transpose(ptq[:, 128 * i:128 * (i + 1)],
                                    qk_stage[:, 128 * i:128 * (i + 1)],
                                    ident[:])
            qa_ = qkT_all[:]
            dst = AP(qa_.tensor, qa_.offset + 128 * t,
                     [qa_.ap[0], [N, 4], [1, 128]])
            nc.scalar.copy(dst, ptq[:])

    def emit_attn(Q, psS, psC):
        qcol = slice(512 * Q, 512 * (Q + 1))
        for h in range(NH):
            g, off = divmod(h, 2)
            row = slice(64 * off, 64 * off + 64)
            npair = 2 * Q + 2
            pctx = psC.tile([65, 512], F32, tag="ctx")
            pts = {}

            def emit_st_pair(p):
                pst = psS.tile([128, 1024], F32, tag="st",
                               name=f"st{Q}_{h}_{p}")
                for s in range(2):
                    j = 2 * p + s
                    nc.tensor.matmul(
                        pst[:, 512 * s:512 * (s + 1)],
                        kT[g][row, 128 * j:128 * (j + 1)],
                        qT[g][row, qcol], start=True, stop=True)
                pt = ptp.tile([128, 1024], cfg.dt_v, tag="pt",
                              name=f"pt{Q}_{h}_{p}")
                nc.scalar.activation(pt[:], pst[:], AFT.Exp)
                o = 2 * p - 4 * Q
                if o >= 0:
                    nc.vector.tensor_mul(
                        pt[:], pt[:], masks_sb[:, 512 * o:512 * (o + 2)])
                pts[p] = pt

            def emit_ctx_pair(p):
                pt = pts.pop(p)
                for s in range(2):
                    j = 2 * p + s
                    nc.tensor.matmul(
                        pctx[:],
                        vt[:, 65 * (NH * j + h):65 * (NH * j + h) + 65],
                        pt[:, 512 * s:512 * (s + 1)],
                        start=(j == 0), stop=(j == 2 * npair - 1))

            emit_st_pair(0)
            for p in range(1, npair):
                emit_st_pair(p)
                emit_ctx_pair(p - 1)
            emit_ctx_pair(npair - 1)

            den_sb = scratch.tile([1, 512], F32, tag="den_sb")
            nc.vector.tensor_copy(den_sb[:], pctx[64:65, :])
            recip1 = scratch.tile([1, 512], F32, tag="recip1")
            rscr = scratch.tile([1, 512], F32, tag="rscr")
            nc.vector.reciprocal_approx_accurate(recip1[:], den_sb[:], rscr[:])
            recip = scratch.tile([64, 512], F32, tag="recip")
            nc.gpsimd.partition_broadcast(recip[:], recip1[:])
            nc.vector.tensor_mul(ctxT[g][row, qcol], pctx[0:64, :], recip[:])

    def emit_outproj(Q, psO):
        qcol = slice(512 * Q, 512 * (Q + 1))
        for m in range(DC):
            po = psO.tile([128, 512], F32, tag="po")
            for r in range(2):
                nc.tensor.matmul(
                    po[:],
                    wo_sb[:, D * r + 128 * m:D * r + 128 * (m + 1)],
                    ctxT[r][:, qcol], start=(r == 0), stop=(r == 1))
            ob = scratch.tile([128, 512], F32, tag="ob")
            nc.scalar.copy(ob[:], po[:])
            nc.sync.dma_start(outT[128 * m:128 * (m + 1), qcol], ob[:])

    # ---- interleaved schedule: A-group 0, Q0+Q1, A-group 1, Q2+Q3 ----
    with tc.tile_pool(name="psA0", bufs=2, space="PSUM") as psA, \
         tc.tile_pool(name="psT0", bufs=2, space="PSUM") as psT:
        emit_groupA(0, psA, psT)
    with tc.tile_pool(name="psS0", bufs=2, space="PSUM") as psS, \
         tc.tile_pool(name="psC0", bufs=2, space="PSUM") as psC, \
         tc.tile_pool(name="psO0", bufs=2, space="PSUM") as psO:
        emit_attn(0, psS, psC)
        emit_attn(1, psS, psC)
        emit_outproj(0, psO)
        emit_outproj(1, psO)
    with tc.tile_pool(name="psA1", bufs=2, space="PSUM") as psA, \
         tc.tile_pool(name="psT1", bufs=2, space="PSUM") as psT:
        emit_groupA(1, psA, psT)
    with tc.tile_pool(name="psS1", bufs=2, space="PSUM") as psS, \
         tc.tile_pool(name="psC1", bufs=2, space="PSUM") as psC, \
         tc.tile_pool(name="psO1", bufs=2, space="PSUM") as psO:
        emit_attn(2, psS, psC)
        emit_attn(3, psS, psC)
        emit_outproj(2, psO)
        emit_outproj(3, psO)


# ---------------------------------------------------------------------------
# host side
# ---------------------------------------------------------------------------

_CACHE = {}


def _get_nc(cfg):
    key = cfg.variant
    if key not in _CACHE:
        _CACHE[key] = build_nc(cfg)
    return _CACHE[key]


def _host_tables(q_ln_scale, k_ln_scale, per_dim_scale):
    frac = 2.0 * np.arange(HALF, dtype=np.float32) / HD
    ts = (MAX_TIMESCALE ** frac).astype(np.float32)
    pos = np.arange(N, dtype=np.float32)
    sinu = pos[:, None] / ts[None, :]
    SIN = np.sin(sinu).astype(np.float32)
    COS = np.cos(sinu).astype(np.float32)
    qs = (LOG2_E / np.sqrt(np.float32(HD))
          * np.logaddexp(0.0, per_dim_scale.astype(np.float64))).astype(np.float32)
    qscale = (q_ln_scale * qs).astype(np.float32)
    kscale = k_ln_scale.astype(np.float32)

    # combined table [N, 256]: func f in {cosA,sinA,cosB,sinB} at cols
    # [64f:64f+64], q-scaled half at +0:32, k-scaled at +32:64
    blocks = []
    for base, half in ((COS, slice(0, HALF)), (SIN, slice(0, HALF)),
                       (COS, slice(HALF, HD)), (SIN, slice(HALF, HD))):
        blocks.append(base * qscale[None, half])
        blocks.append(base * kscale[None, half])
    return np.concatenate(blocks, axis=1).astype(np.float32)


def _host_masks(dt_np_):
    # mask_o[r, c] = 1 if c >= r + 128*o  (S.T block: rows k, cols q)
    r = np.arange(128)[:, None]
    c = np.arange(512)[None, :]
    ms = [(c >= r + 128 * o).astype(np.float32) for o in range(4)]
    return np.concatenate(ms, axis=1).astype(dt_np_)


def kernel(**inputs):
    from concourse.bass_utils import run_bass_kernel_spmd

    cfg = Cfg(VARIANT)
    nc = _get_nc(cfg)

    x = np.asarray(inputs["inputs_q"], dtype=np.float32)
    wq = np.asarray(inputs["wq"], dtype=np.float32)
    wk = np.asarray(inputs["wk"], dtype=np.float32)
    wv = np.asarray(inputs["wv"], dtype=np.float32)
    wo = np.asarray(inputs["wo"], dtype=np.float32)

    ctab = _host_tables(np.asarray(inputs["q_ln_scale"], np.float32),
                        np.asarray(inputs["k_ln_scale"], np.float32),
                        np.asarray(inputs["per_dim_scale"], np.float32))
    dtw_np = _dt_np(cfg.dt_w)
    dtx_np = _dt_np(cfg.dt_x)
    masks = _host_masks(_dt_np(cfg.dt_v))

    in_maps = []
    for c in range(8):
        b, g = divmod(c, 4)
        hs = slice(NH * g, NH * (g + 1))
        wqkv_c = np.concatenate(
            [wq[:, hs, :].reshape(D, NH * HD),
             wk[:, hs, :].reshape(D, NH * HD),
             wv[:, hs, :].reshape(D, NH * HD)], axis=1)
        in_maps.append({
            "x": np.ascontiguousarray(x[b]).astype(dtx_np),
            "wqkv": np.ascontiguousarray(wqkv_c).astype(dtw_np),
            "wo": np.ascontiguousarray(wo[hs].reshape(NH * HD, D)).astype(
                _dt_np(cfg.dt_wo)),
            "ctab": ctab, "masks": masks,
            "ident": np.eye(128, dtype=_dt_np(cfg.dt_mm)),
            "ones": np.ones((128, NH * HD), dtype=_dt_np(cfg.dt_v)),
        })

    trace = os.environ.get("MHA_TRACE", "0") == "1"
    res = run_bass_kernel_spmd(nc, in_maps, list(range(8)), trace=trace)
    if trace:
        kernel.last_exec_time_ns = res.exec_time_ns
        kernel.last_results = res

    out = np.zeros((B, N, D), dtype=np.float32)
    for c in range(8):
        out[c // 4] += res.results[c]["outT"].T
    return out
